# revision 17
# baseline (speedup 1.0000x reference)
"""PointPillars encoder on 8 Trainium2 NeuronCores (Bass/Tile).

Sharding: data-parallel over batch (2) x image-width quarters (4) = 8 cores.
Core c: batch c//4, width-quarter c%4 of the NY*NX = 35200-wide pseudo-image.

Host does integer-only index prep (batchnorm folding, voxelization,
last-write-wins winner resolution, gather-index maps). All FP math and all
heavy data movement run on device:
  TC1: pointwise feature extractor (3->32->64) over compact winner points ->
       fp16 feature table [32768, 128] in DRAM (256B rows).
  TC2: per width-tile, transpose-mode dma_gather materializes the image slice
       channel-major in SBUF (the dense-grid scatter, realized as a gather);
       conv0(s2) + 2 residual blocks + 2 maxpools as shifted matmuls.
"""
import sys
sys.path.insert(0, '/opt/trn_rl_repo')
import os
import numpy as np

PC_LO = np.array([0.0, -40.0, -3.0], np.float32)
VOXEL = np.float32(0.4)
NX, NY, NZ = 176, 200, 10
WIMG = NY * NX
NV = NZ * WIMG
EPS = 1e-5

TBL = 32768
EMPTY_ID = TBL - 2      # row = FE((0,0,0)) = empty-voxel value
ZERO_ID = TBL - 1       # row = zeros = out-of-image padding
NCORES = 8
NQ = 4
NT = 4                  # width-tiles per core
IMGW = 2304             # img tile width (18*128)
C0WP = 1120             # conv0-out tile padded width (valid 1112)
P1WP = 560              # r1pool-out tile padded width (valid 554)
FW = 275                # final cols per wtile
GN = IMGW

LAST_EXEC_NS = None


def _wrap_idx(idx_row):
    n = idx_row.shape[0]
    t = idx_row.reshape(n // 16, 16).T
    return np.tile(t, (8, 1)).astype(np.int16)


def _fold_conv(w, b, bn):
    s = (np.asarray(bn['g'], np.float32) / np.sqrt(np.asarray(bn['v'], np.float32) + EPS))
    t = np.asarray(bn['b'], np.float32) - np.asarray(bn['m'], np.float32) * s
    w = np.asarray(w, np.float32) * s.reshape((-1,) + (1,) * (np.asarray(w).ndim - 1))
    b = np.asarray(b, np.float32) * s + t
    return w, b


def _prep_host(points, params):
    pts = np.asarray(points, np.float32)
    B, N, _ = pts.shape

    w1, b1 = _fold_conv(params['fe']['w1'], params['fe']['b1'], params['fe']['bn1'])
    w2, b2 = _fold_conv(params['fe']['w2'], params['fe']['b2'], params['fe']['bn2'])
    c0w, c0b = _fold_conv(params['bb']['c0_w'], params['bb']['c0_b'], params['bb']['bn0'])
    r1c1w, r1c1b = _fold_conv(params['r1']['c1_w'], params['r1']['c1_b'], params['r1']['bn1'])
    r1c2w, r1c2b = _fold_conv(params['r1']['c2_w'], params['r1']['c2_b'], params['r1']['bn2'])
    r1scw, r1scb = _fold_conv(params['r1']['sc_w'], params['r1']['sc_b'], params['r1']['sc_bn'])
    r2c1w, r2c1b = _fold_conv(params['r2']['c1_w'], params['r2']['c1_b'], params['r2']['bn1'])
    r2c2w, r2c2b = _fold_conv(params['r2']['c2_w'], params['r2']['c2_b'], params['r2']['bn2'])
    r2scw, r2scb = _fold_conv(params['r2']['sc_w'], params['r2']['sc_b'], params['r2']['sc_bn'])

    def taps_sb(w):  # [O, I, 3, 3] -> [I, 9, O] fp16 (lhsT per tap, partition-major I)
        return np.ascontiguousarray(
            np.asarray(w).transpose(1, 2, 3, 0).reshape(w.shape[1], 9, w.shape[0])
        ).astype(np.float16)

    r2c2_t = r2c2w.transpose(2, 3, 1, 0).reshape(9, 256, 256)   # [tap, i, o]
    wd = {
        'w1t': np.ascontiguousarray(w1.T, dtype=np.float32),     # [3, 32]
        'b1': np.ascontiguousarray(b1.reshape(32, 1), dtype=np.float32),
        'w2t': np.ascontiguousarray(w2.T, dtype=np.float32),     # [32, 64]
        'b2t': np.tile(b2.reshape(1, 64), (128, 1)).astype(np.float32),
        'c0w': taps_sb(c0w),                                     # [64, 9, 64]
        'c0b': np.ascontiguousarray(c0b.reshape(64, 1), dtype=np.float32),
        'r1c1w': taps_sb(r1c1w),                                 # [64, 9, 128]
        'r1c1b': np.ascontiguousarray(r1c1b.reshape(128, 1), dtype=np.float32),
        'r1c2w': taps_sb(r1c2w),                                 # [128, 9, 128]
        'r1ob': np.ascontiguousarray((r1c2b + r1scb).reshape(128, 1), dtype=np.float32),
        'r1scw': np.ascontiguousarray(r1scw[:, :, 0, 0].T).astype(np.float16),  # [64, 128]
        'r2c1w': taps_sb(r2c1w),                                 # [128, 9, 256]
        'r2c1b': np.ascontiguousarray(r2c1b.reshape(2, 128).T, dtype=np.float32),  # [128, 2]
        'r2c2w': np.ascontiguousarray(
            r2c2_t.reshape(9, 2, 128, 256).transpose(2, 0, 1, 3)).astype(np.float16),  # [128,9,2,256]
        'r2ob': np.ascontiguousarray(
            (r2c2b + r2scb).reshape(2, 128).T, dtype=np.float32),  # [128, 2]
        'r2scw': np.ascontiguousarray(r2scw[:, :, 0, 0].T).astype(np.float16),  # [128, 256]
    }

    iv = ((pts - PC_LO.reshape(1, 1, 3)) / VOXEL).astype(np.int32)
    iv = np.clip(iv, 0, np.array([NX - 1, NY - 1, NZ - 1], np.int32))
    flat = (iv[..., 2] * NY + iv[..., 1]) * NX + iv[..., 0]

    per_core = []
    for b in range(B):
        last = np.full(NV, -1, np.int64)
        np.maximum.at(last, flat[b], np.arange(N, dtype=np.int64))
        w_of_v = np.arange(NV, dtype=np.int64) % WIMG
        for q in range(NQ):
            lo_c, hi_c = 8800 * q - 13, 8800 * q + 8812
            in_reg = (last >= 0) & (w_of_v >= max(lo_c, 0)) & (w_of_v < min(hi_c, WIMG))
            vids = np.nonzero(in_reg)[0]
            K = len(vids)
            assert K <= TBL - 2, f"winner count {K} exceeds table"
            cid = np.full(NV, EMPTY_ID, np.int32)
            cid[vids] = np.arange(K, dtype=np.int32)
            ptsT = np.zeros((3, TBL), np.float32)
            ptsT[:, :K] = pts[b, last[vids]].T
            sel = np.full((NT, NZ, IMGW), ZERO_ID, np.int32)
            for t in range(NT):
                w0 = 8800 * q + 2200 * t - 13
                cols = np.arange(w0, w0 + 2225)
                valid = (cols >= 0) & (cols < WIMG)
                vcols = cols[valid]
                vpos = np.nonzero(valid)[0]
                for z in range(NZ):
                    sel[t, z, vpos] = cid[z * WIMG + vcols]
            selw = np.concatenate(
                [_wrap_idx(sel[t, z]) for t in range(NT) for z in range(NZ)], axis=1)
            m0 = np.zeros((128, NT * C0WP), np.float16)
            mp = np.zeros((128, NT * P1WP), np.float16)
            for t in range(NT):
                a0 = 4400 * q + 1100 * t - 6
                j = np.arange(C0WP)
                m0[:, t * C0WP:(t + 1) * C0WP] = (
                    (a0 + j >= 0) & (a0 + j < 17600) & (j < 1112)).astype(np.float16)
                p0 = 2200 * q + 550 * t - 2
                j = np.arange(P1WP)
                mp[:, t * P1WP:(t + 1) * P1WP] = (
                    (p0 + j >= 0) & (p0 + j < 8800) & (j < 554)).astype(np.float16)
            per_core.append({'ptsT': ptsT, 'selw': selw, 'm0': m0, 'mp': mp})
    return per_core, wd


def _build_bass():
    import concourse.bacc as bacc
    import concourse.mybir as mybir
    from concourse.tile import TileContext

    f32, f16, i16 = mybir.dt.float32, mybir.dt.float16, mybir.dt.int16
    RELU = mybir.ActivationFunctionType.Relu
    MAX = mybir.AluOpType.max
    ADD = mybir.AluOpType.add
    MULT = mybir.AluOpType.mult

    nc = bacc.Bacc("TRN2", target_bir_lowering=False, debug=False,
                   num_devices=NCORES)
    D = {}
    def din(name, shape, dt):
        D[name] = nc.dram_tensor(name, shape, dt, kind="ExternalInput")
        return D[name]

    ptsT_d = din("ptsT", [3, TBL], f32)
    selw_d = din("selw", [128, NT * NZ * (GN // 16)], i16)
    m0_d = din("m0", [128, NT * C0WP], f16)
    mp_d = din("mp", [128, NT * P1WP], f16)
    w1t_d = din("w1t", [3, 32], f32)
    b1_d = din("b1", [32, 1], f32)
    w2t_d = din("w2t", [32, 64], f32)
    b2t_d = din("b2t", [128, 64], f32)
    c0w_d = din("c0w", [64, 9, 64], f16)
    c0b_d = din("c0b", [64, 1], f32)
    r1c1w_d = din("r1c1w", [64, 9, 128], f16)
    r1c1b_d = din("r1c1b", [128, 1], f32)
    r1c2w_d = din("r1c2w", [128, 9, 128], f16)
    r1ob_d = din("r1ob", [128, 1], f32)
    r1scw_d = din("r1scw", [64, 128], f16)
    r2c1w_d = din("r2c1w", [128, 9, 256], f16)
    r2c1b_d = din("r2c1b", [128, 2], f32)
    r2c2w_d = din("r2c2w", [128, 9, 2, 256], f16)
    r2ob_d = din("r2ob", [128, 2], f32)
    r2scw_d = din("r2scw", [128, 256], f16)
    out_d = nc.dram_tensor("out", [256, NT * FW], f32, kind="ExternalOutput")
    tbl_d = nc.dram_tensor("tbl", [TBL, 128], f16, kind="Internal")

    # ---- TC1: feature extractor -> DRAM table ------------------------------
    with TileContext(nc) as tc:
        with tc.tile_pool(name="fe", bufs=1) as fp, \
             tc.tile_pool(name="feb", bufs=4) as fb, \
             tc.tile_pool(name="fps", bufs=4, space="PSUM") as pp:
            QT = TBL // 4
            w1t = fp.tile([3, 32], f32)
            b1 = fp.tile([32, 1], f32)
            w2t = fp.tile([32, 64], f32)
            b2t = fp.tile([128, 64], f32)
            nc.sync.dma_start(w1t[:, :], w1t_d[:, :])
            nc.sync.dma_start(b1[:, :], b1_d[:, :])
            nc.sync.dma_start(w2t[:, :], w2t_d[:, :])
            nc.sync.dma_start(b2t[:, :], b2t_d[:, :])
            for h in range(4):
                ptsT = fp.tile([3, QT], f32, tag="ptsT")
                x1 = fp.tile([32, QT], f32, tag="x1")
                nc.sync.dma_start(ptsT[:, :], ptsT_d[:, h * QT:(h + 1) * QT])
                for j in range(QT // 512):
                    ps = pp.tile([32, 512], f32, tag="ps1")
                    nc.tensor.matmul(ps[:, :], w1t[:, :], ptsT[:, j * 512:(j + 1) * 512],
                                     start=True, stop=True)
                    nc.scalar.activation(x1[:, j * 512:(j + 1) * 512], ps[:, :],
                                         RELU, bias=b1[:, :], scale=1.0)
                for blk in range(QT // 128):
                    gblk = h * (QT // 128) + blk
                    ps2 = pp.tile([128, 64], f32, tag="ps2")
                    nc.tensor.matmul(ps2[:, :], x1[:, blk * 128:(blk + 1) * 128],
                                     w2t[:, :], start=True, stop=True)
                    s2 = fb.tile([128, 64], f32, tag="s2")
                    nc.vector.tensor_tensor(s2[:, :], ps2[:, :], b2t[:, :], op=ADD)
                    row = fb.tile([128, 64], f16, tag="row")
                    nc.scalar.activation(row[:, :], s2[:, :], RELU)
                    nc.sync.dma_start(tbl_d[gblk * 128:(gblk + 1) * 128, 0:64], row[:, :])
            zr = fb.tile([1, 128], f16, tag="zr")
            nc.vector.memset(zr[:, :], 0)
            nc.sync.dma_start(tbl_d[ZERO_ID:ZERO_ID + 1, :], zr[:, :])

    # ---- TC2: gather image + conv stack ------------------------------------
    with TileContext(nc) as tc:
        with tc.tile_pool(name="cw", bufs=1) as wp, \
             tc.tile_pool(name="img", bufs=2) as ip, \
             tc.tile_pool(name="act", bufs=1) as ap_, \
             tc.tile_pool(name="sm", bufs=2) as sp, \
             tc.tile_pool(name="cps", bufs=4, space="PSUM") as pp:
            selw = wp.tile([128, NT * NZ * (GN // 16)], i16)
            nc.sync.dma_start(selw[:, :], selw_d[:, :])
            m0 = wp.tile([128, NT * C0WP], f16)
            mp = wp.tile([128, NT * P1WP], f16)
            c0w = wp.tile([64, 9, 64], f16)
            c0b = wp.tile([64, 1], f32)
            r1c1w = wp.tile([64, 9, 128], f16)
            r1c1b = wp.tile([128, 1], f32)
            r1c2w = wp.tile([128, 9, 128], f16)
            r1ob = wp.tile([128, 1], f32)
            r1scw = wp.tile([64, 128], f16)
            r2c1w = wp.tile([128, 9, 256], f16)
            r2c1b = wp.tile([128, 2], f32)
            r2c2w = wp.tile([128, 9, 2, 256], f16)
            r2ob = wp.tile([128, 2], f32)
            r2scw = wp.tile([128, 256], f16)
            nc.sync.dma_start(m0[:, :], m0_d[:, :])
            nc.sync.dma_start(mp[:, :], mp_d[:, :])
            nc.sync.dma_start(c0w[:, :, :], c0w_d[:, :, :])
            nc.sync.dma_start(c0b[:, :], c0b_d[:, :])
            nc.sync.dma_start(r1c1w[:, :, :], r1c1w_d[:, :, :])
            nc.sync.dma_start(r1c1b[:, :], r1c1b_d[:, :])
            nc.sync.dma_start(r1c2w[:, :, :], r1c2w_d[:, :, :])
            nc.sync.dma_start(r1ob[:, :], r1ob_d[:, :])
            nc.sync.dma_start(r1scw[:, :], r1scw_d[:, :])
            nc.sync.dma_start(r2c1w[:, :, :], r2c1w_d[:, :, :])
            nc.sync.dma_start(r2c1b[:, :], r2c1b_d[:, :])
            nc.sync.dma_start(r2c2w[:, :, :, :], r2c2w_d[:, :, :, :])
            nc.sync.dma_start(r2ob[:, :], r2ob_d[:, :])
            nc.sync.dma_start(r2scw[:, :], r2scw_d[:, :])

            def chunks(width, cmax=512):
                c = 0
                while c < width:
                    n = min(cmax, width - c)
                    yield c, n
                    c += n

            for t in range(NT):
                img = ip.tile([128, NZ, IMGW], f16, tag="img")
                for z in range(NZ):
                    nc.gpsimd.dma_gather(
                        out_ap=img[:, z:z + 1, :], in_ap=tbl_d[:, :],
                        idxs_ap=selw[:, (t * NZ + z) * (GN // 16):(t * NZ + z + 1) * (GN // 16)],
                        num_idxs=GN, num_idxs_reg=GN, elem_size=128,
                        transpose=True, single_packet=False)

                # conv0 s2 3x3 64->64, relu, mask -> c0o [64, 5, C0WP]
                c0o = ap_.tile([64, 5, C0WP], f16, tag="c0o")
                for ho in range(5):
                    for c, n in chunks(1112):
                        ps = pp.tile([64, 512], f32, tag="ps")
                        taps = [(dy, dx) for dy in (-1, 0, 1) for dx in (-1, 0, 1)
                                if 0 <= 2 * ho + dy < NZ]
                        for i, (dy, dx) in enumerate(taps):
                            st = 1 + dx + 2 * c
                            nc.tensor.matmul(
                                ps[:, :n], c0w[:, 3 * (dy + 1) + (dx + 1), :],
                                img[0:64, 2 * ho + dy, st:st + 2 * n:2],
                                start=(i == 0), stop=(i == len(taps) - 1))
                        nc.scalar.activation(c0o[:, ho, c:c + n], ps[:, :n],
                                             RELU, bias=c0b[:, :], scale=1.0)
                    nc.vector.tensor_tensor(c0o[:, ho, :], c0o[:, ho, :],
                                            m0[0:64, t * C0WP:(t + 1) * C0WP], op=MULT)

                # r1 h1 = relu(c1(c0o)), valid j in [1, 1111)
                h1 = ap_.tile([128, 5, C0WP], f16, tag="h1")
                for ho in range(5):
                    for c, n in chunks(1110):
                        ps = pp.tile([128, 512], f32, tag="ps")
                        taps = [(dy, dx) for dy in (-1, 0, 1) for dx in (-1, 0, 1)
                                if 0 <= ho + dy < 5]
                        for i, (dy, dx) in enumerate(taps):
                            st = 1 + c + dx
                            nc.tensor.matmul(
                                ps[:, :n], r1c1w[:, 3 * (dy + 1) + (dx + 1), :],
                                c0o[0:64, ho + dy, st:st + n],
                                start=(i == 0), stop=(i == len(taps) - 1))
                        nc.scalar.activation(h1[:, ho, 1 + c:1 + c + n], ps[:, :n],
                                             RELU, bias=r1c1b[:, :], scale=1.0)
                    nc.vector.tensor_tensor(h1[:, ho, :], h1[:, ho, :],
                                            m0[:, t * C0WP:(t + 1) * C0WP], op=MULT)

                # r1 out = relu(c2(h1) + sc(c0o)), valid j in [2, 1110)
                r1o = ap_.tile([128, 5, C0WP], f16, tag="r1o")
                for ho in range(5):
                    for c, n in chunks(1108):
                        ps = pp.tile([128, 512], f32, tag="ps")
                        taps = [(dy, dx) for dy in (-1, 0, 1) for dx in (-1, 0, 1)
                                if 0 <= ho + dy < 5]
                        for i, (dy, dx) in enumerate(taps):
                            st = 2 + c + dx
                            nc.tensor.matmul(
                                ps[:, :n], r1c2w[:, 3 * (dy + 1) + (dx + 1), :],
                                h1[:, ho + dy, st:st + n],
                                start=(i == 0), stop=False)
                        nc.tensor.matmul(ps[:, :n], r1scw[:, :],
                                         c0o[0:64, ho, 2 + c:2 + c + n],
                                         start=False, stop=True)
                        nc.scalar.activation(r1o[:, ho, 2 + c:2 + c + n], ps[:, :n],
                                             RELU, bias=r1ob[:, :], scale=1.0)

                # maxpool1 -> p1 [128, 2, P1WP] valid [0,554), mask
                p1 = ap_.tile([128, 2, P1WP], f16, tag="p1")
                tmpa = sp.tile([128, P1WP], f16, tag="tmpa")
                tmpb = sp.tile([128, P1WP], f16, tag="tmpb")
                for hp in range(2):
                    nc.vector.tensor_tensor(tmpa[:, 0:554],
                                            r1o[:, 2 * hp, 2:1110:2],
                                            r1o[:, 2 * hp, 3:1111:2], op=MAX)
                    nc.vector.tensor_tensor(tmpb[:, 0:554],
                                            r1o[:, 2 * hp + 1, 2:1110:2],
                                            r1o[:, 2 * hp + 1, 3:1111:2], op=MAX)
                    nc.vector.tensor_tensor(p1[:, hp, 0:554], tmpa[:, 0:554],
                                            tmpb[:, 0:554], op=MAX)
                    nc.vector.tensor_tensor(p1[:, hp, :], p1[:, hp, :],
                                            mp[:, t * P1WP:(t + 1) * P1WP], op=MULT)

                # r2 h2 = relu(c1(p1)), valid [1, 553), two M-halves
                h2 = [ap_.tile([128, 2, P1WP], f16, tag=f"h2{m}", name=f"h2{m}_{t}") for m in range(2)]
                for m in range(2):
                    for ho in range(2):
                        for c, n in chunks(552):
                            ps = pp.tile([128, 512], f32, tag="ps")
                            taps = [(dy, dx) for dy in (-1, 0, 1) for dx in (-1, 0, 1)
                                    if 0 <= ho + dy < 2]
                            for i, (dy, dx) in enumerate(taps):
                                st = 1 + c + dx
                                nc.tensor.matmul(
                                    ps[:, :n],
                                    r2c1w[:, 3 * (dy + 1) + (dx + 1), 128 * m:128 * (m + 1)],
                                    p1[:, ho + dy, st:st + n],
                                    start=(i == 0), stop=(i == len(taps) - 1))
                            nc.scalar.activation(h2[m][:, ho, 1 + c:1 + c + n], ps[:, :n],
                                                 RELU, bias=r2c1b[:, m:m + 1], scale=1.0)
                        nc.vector.tensor_tensor(h2[m][:, ho, :], h2[m][:, ho, :],
                                                mp[:, t * P1WP:(t + 1) * P1WP], op=MULT)

                # r2 out = relu(c2(h2) + sc(p1)), valid [2, 552)
                r2o = [ap_.tile([128, 2, P1WP], f16, tag=f"r2o{m}", name=f"r2o{m}_{t}") for m in range(2)]
                for m in range(2):
                    for ho in range(2):
                        for c, n in chunks(550):
                            ps = pp.tile([128, 512], f32, tag="ps")
                            taps = [(dy, dx) for dy in (-1, 0, 1) for dx in (-1, 0, 1)
                                    if 0 <= ho + dy < 2]
                            first = True
                            for (dy, dx) in taps:
                                for kh in range(2):
                                    st = 2 + c + dx
                                    nc.tensor.matmul(
                                        ps[:, :n],
                                        r2c2w[:, 3 * (dy + 1) + (dx + 1), kh, 128 * m:128 * (m + 1)],
                                        h2[kh][:, ho + dy, st:st + n],
                                        start=first, stop=False)
                                    first = False
                            nc.tensor.matmul(ps[:, :n], r2scw[:, 128 * m:128 * (m + 1)],
                                             p1[:, ho, 2 + c:2 + c + n],
                                             start=False, stop=True)
                            nc.scalar.activation(r2o[m][:, ho, 2 + c:2 + c + n], ps[:, :n],
                                                 RELU, bias=r2ob[:, m:m + 1], scale=1.0)

                # maxpool2 -> out [2x128, FW] f32
                for m in range(2):
                    oa = sp.tile([128, FW], f16, tag=f"oa{m}")
                    ob = sp.tile([128, FW], f16, tag=f"ob{m}")
                    oc = sp.tile([128, FW], f32, tag=f"oc{m}")
                    nc.vector.tensor_tensor(oa[:, :], r2o[m][:, 0, 2:552:2],
                                            r2o[m][:, 0, 3:553:2], op=MAX)
                    nc.vector.tensor_tensor(ob[:, :], r2o[m][:, 1, 2:552:2],
                                            r2o[m][:, 1, 3:553:2], op=MAX)
                    nc.vector.tensor_tensor(oc[:, :], oa[:, :], ob[:, :], op=MAX)
                    nc.sync.dma_start(out_d[128 * m:128 * (m + 1), t * FW:(t + 1) * FW],
                                      oc[:, :])
    nc.finalize()
    return nc


_NC_CACHE = None


def kernel(points, params):
    global _NC_CACHE, LAST_EXEC_NS
    from concourse.bass_utils import run_bass_kernel_spmd

    per_core, wd = _prep_host(points, params)
    if _NC_CACHE is None:
        _NC_CACHE = _build_bass()
    nc = _NC_CACHE
    in_maps = []
    for c in range(NCORES):
        m = dict(wd)
        m.update(per_core[c])
        in_maps.append({k: np.ascontiguousarray(v) for k, v in m.items()})
    trace = bool(int(os.environ.get("PP_TRACE", "0")))
    res = run_bass_kernel_spmd(nc, in_maps, core_ids=list(range(NCORES)),
                               trace=trace)
    LAST_EXEC_NS = res.exec_time_ns
    B = np.asarray(points).shape[0]
    y = np.zeros((B, 256, 1, 4400), np.float32)
    for c in range(NCORES):
        b, q = c // NQ, c % NQ
        y[b, :, 0, 1100 * q:1100 * (q + 1)] = res.results[c]['out']
    return y


# revision 18
# speedup vs baseline: 1.6473x; 1.6473x over previous
"""PointPillars encoder on 8 Trainium2 NeuronCores (Bass/Tile).

Sharding: data-parallel over batch (2) x image-width quarters (4) = 8 cores.
Core c: batch c//4, width-quarter c%4 of the NY*NX = 35200-wide pseudo-image.

Host does integer-only index prep (batchnorm folding, voxelization,
last-write-wins winner resolution, gather-index maps). All FP math and all
heavy data movement run on device:
  TC1: pointwise feature extractor (3->32->64) over compact winner points ->
       fp16 feature table [32768, 128] in DRAM (256B rows).
  TC2: per width-tile, transpose-mode dma_gather materializes the image slice
       channel-major in SBUF (the dense-grid scatter, realized as a gather);
       conv0(s2) + 2 residual blocks + 2 maxpools as shifted matmuls.
"""
import sys
sys.path.insert(0, '/opt/trn_rl_repo')
import os
import numpy as np

PC_LO = np.array([0.0, -40.0, -3.0], np.float32)
VOXEL = np.float32(0.4)
NX, NY, NZ = 176, 200, 10
WIMG = NY * NX
NV = NZ * WIMG
EPS = 1e-5

TBL = 32768
EMPTY_ID = TBL - 2      # row = FE((0,0,0)) = empty-voxel value
ZERO_ID = TBL - 1       # row = zeros = out-of-image padding
NCORES = 8
NQ = 4
NT = 4                  # width-tiles per core
IMGW = 2304             # img tile width (18*128)
C0WP = 1120             # conv0-out tile padded width (valid 1112)
P1WP = 560              # r1pool-out tile padded width (valid 554)
FW = 275                # final cols per wtile
GN = IMGW

LAST_EXEC_NS = None


def _wrap_idx(idx_row):
    n = idx_row.shape[0]
    t = idx_row.reshape(n // 16, 16).T
    return np.tile(t, (8, 1)).astype(np.int16)


def _fold_conv(w, b, bn):
    s = (np.asarray(bn['g'], np.float32) / np.sqrt(np.asarray(bn['v'], np.float32) + EPS))
    t = np.asarray(bn['b'], np.float32) - np.asarray(bn['m'], np.float32) * s
    w = np.asarray(w, np.float32) * s.reshape((-1,) + (1,) * (np.asarray(w).ndim - 1))
    b = np.asarray(b, np.float32) * s + t
    return w, b


def _prep_host(points, params):
    pts = np.asarray(points, np.float32)
    B, N, _ = pts.shape

    w1, b1 = _fold_conv(params['fe']['w1'], params['fe']['b1'], params['fe']['bn1'])
    w2, b2 = _fold_conv(params['fe']['w2'], params['fe']['b2'], params['fe']['bn2'])
    c0w, c0b = _fold_conv(params['bb']['c0_w'], params['bb']['c0_b'], params['bb']['bn0'])
    r1c1w, r1c1b = _fold_conv(params['r1']['c1_w'], params['r1']['c1_b'], params['r1']['bn1'])
    r1c2w, r1c2b = _fold_conv(params['r1']['c2_w'], params['r1']['c2_b'], params['r1']['bn2'])
    r1scw, r1scb = _fold_conv(params['r1']['sc_w'], params['r1']['sc_b'], params['r1']['sc_bn'])
    r2c1w, r2c1b = _fold_conv(params['r2']['c1_w'], params['r2']['c1_b'], params['r2']['bn1'])
    r2c2w, r2c2b = _fold_conv(params['r2']['c2_w'], params['r2']['c2_b'], params['r2']['bn2'])
    r2scw, r2scb = _fold_conv(params['r2']['sc_w'], params['r2']['sc_b'], params['r2']['sc_bn'])

    def taps_sb(w):  # [O, I, 3, 3] -> [I, 9, O] fp16 (lhsT per tap, partition-major I)
        return np.ascontiguousarray(
            np.asarray(w).transpose(1, 2, 3, 0).reshape(w.shape[1], 9, w.shape[0])
        ).astype(np.float16)

    r2c2_t = r2c2w.transpose(2, 3, 1, 0).reshape(9, 256, 256)   # [tap, i, o]
    wd = {
        'w1t': np.ascontiguousarray(w1.T, dtype=np.float32),     # [3, 32]
        'b1': np.ascontiguousarray(b1.reshape(32, 1), dtype=np.float32),
        'w2t': np.ascontiguousarray(w2.T, dtype=np.float32),     # [32, 64]
        'b2t': np.tile(b2.reshape(1, 64), (128, 1)).astype(np.float32),
        'c0w': taps_sb(c0w),                                     # [64, 9, 64]
        'c0b': np.ascontiguousarray(c0b.reshape(64, 1), dtype=np.float32),
        'r1c1w': taps_sb(r1c1w),                                 # [64, 9, 128]
        'r1c1b': np.ascontiguousarray(r1c1b.reshape(128, 1), dtype=np.float32),
        'r1c2w': taps_sb(r1c2w),                                 # [128, 9, 128]
        'r1ob': np.ascontiguousarray((r1c2b + r1scb).reshape(128, 1), dtype=np.float32),
        'r1scw': np.ascontiguousarray(r1scw[:, :, 0, 0].T).astype(np.float16),  # [64, 128]
        'r2c1w': taps_sb(r2c1w),                                 # [128, 9, 256]
        'r2c1b': np.ascontiguousarray(r2c1b.reshape(2, 128).T, dtype=np.float32),  # [128, 2]
        'r2c2w': np.ascontiguousarray(
            r2c2_t.reshape(9, 2, 128, 256).transpose(2, 0, 1, 3)).astype(np.float16),  # [128,9,2,256]
        'r2ob': np.ascontiguousarray(
            (r2c2b + r2scb).reshape(2, 128).T, dtype=np.float32),  # [128, 2]
        'r2scw': np.ascontiguousarray(r2scw[:, :, 0, 0].T).astype(np.float16),  # [128, 256]
    }

    iv = ((pts - PC_LO.reshape(1, 1, 3)) / VOXEL).astype(np.int32)
    iv = np.clip(iv, 0, np.array([NX - 1, NY - 1, NZ - 1], np.int32))
    flat = (iv[..., 2] * NY + iv[..., 1]) * NX + iv[..., 0]

    per_core = []
    for b in range(B):
        last = np.full(NV, -1, np.int64)
        np.maximum.at(last, flat[b], np.arange(N, dtype=np.int64))
        w_of_v = np.arange(NV, dtype=np.int64) % WIMG
        for q in range(NQ):
            lo_c, hi_c = 8800 * q - 13, 8800 * q + 8812
            in_reg = (last >= 0) & (w_of_v >= max(lo_c, 0)) & (w_of_v < min(hi_c, WIMG))
            vids = np.nonzero(in_reg)[0]
            K = len(vids)
            assert K <= TBL - 2, f"winner count {K} exceeds table"
            cid = np.full(NV, EMPTY_ID, np.int32)
            cid[vids] = np.arange(K, dtype=np.int32)
            ptsT = np.zeros((3, TBL), np.float32)
            ptsT[:, :K] = pts[b, last[vids]].T
            sel = np.full((NT, NZ, IMGW), ZERO_ID, np.int32)
            for t in range(NT):
                w0 = 8800 * q + 2200 * t - 13
                cols = np.arange(w0, w0 + 2225)
                valid = (cols >= 0) & (cols < WIMG)
                vcols = cols[valid]
                vpos = np.nonzero(valid)[0]
                for z in range(NZ):
                    sel[t, z, vpos] = cid[z * WIMG + vcols]
            selw = np.concatenate(
                [_wrap_idx(sel[t, z]) for t in range(NT) for z in range(NZ)], axis=1)
            m0 = np.zeros((128, NT * C0WP), np.float16)
            mp = np.zeros((128, NT * P1WP), np.float16)
            for t in range(NT):
                a0 = 4400 * q + 1100 * t - 6
                j = np.arange(C0WP)
                m0[:, t * C0WP:(t + 1) * C0WP] = (
                    (a0 + j >= 0) & (a0 + j < 17600) & (j < 1112)).astype(np.float16)
                p0 = 2200 * q + 550 * t - 2
                j = np.arange(P1WP)
                mp[:, t * P1WP:(t + 1) * P1WP] = (
                    (p0 + j >= 0) & (p0 + j < 8800) & (j < 554)).astype(np.float16)
            per_core.append({'ptsT': ptsT, 'selw': selw, 'm0': m0, 'mp': mp})
    return per_core, wd


def _build_bass():
    import concourse.bacc as bacc
    import concourse.mybir as mybir
    from concourse.tile import TileContext

    f32, f16, i16 = mybir.dt.float32, mybir.dt.float16, mybir.dt.int16
    RELU = mybir.ActivationFunctionType.Relu
    MAX = mybir.AluOpType.max
    ADD = mybir.AluOpType.add
    MULT = mybir.AluOpType.mult

    nc = bacc.Bacc("TRN2", target_bir_lowering=False, debug=False,
                   num_devices=NCORES)
    D = {}
    def din(name, shape, dt):
        D[name] = nc.dram_tensor(name, shape, dt, kind="ExternalInput")
        return D[name]

    ptsT_d = din("ptsT", [3, TBL], f32)
    selw_d = din("selw", [128, NT * NZ * (GN // 16)], i16)
    m0_d = din("m0", [128, NT * C0WP], f16)
    mp_d = din("mp", [128, NT * P1WP], f16)
    w1t_d = din("w1t", [3, 32], f32)
    b1_d = din("b1", [32, 1], f32)
    w2t_d = din("w2t", [32, 64], f32)
    b2t_d = din("b2t", [128, 64], f32)
    c0w_d = din("c0w", [64, 9, 64], f16)
    c0b_d = din("c0b", [64, 1], f32)
    r1c1w_d = din("r1c1w", [64, 9, 128], f16)
    r1c1b_d = din("r1c1b", [128, 1], f32)
    r1c2w_d = din("r1c2w", [128, 9, 128], f16)
    r1ob_d = din("r1ob", [128, 1], f32)
    r1scw_d = din("r1scw", [64, 128], f16)
    r2c1w_d = din("r2c1w", [128, 9, 256], f16)
    r2c1b_d = din("r2c1b", [128, 2], f32)
    r2c2w_d = din("r2c2w", [128, 9, 2, 256], f16)
    r2ob_d = din("r2ob", [128, 2], f32)
    r2scw_d = din("r2scw", [128, 256], f16)
    out_d = nc.dram_tensor("out", [256, NT * FW], f32, kind="ExternalOutput")
    tbl_d = nc.dram_tensor("tbl", [TBL, 128], f16, kind="Internal")

    # ---- TC1: feature extractor -> DRAM table ------------------------------
    with TileContext(nc) as tc:
        with tc.tile_pool(name="fe", bufs=1) as fp, \
             tc.tile_pool(name="feb", bufs=4) as fb, \
             tc.tile_pool(name="fps", bufs=4, space="PSUM") as pp:
            QT = TBL // 4
            w1t = fp.tile([3, 32], f32)
            b1 = fp.tile([32, 1], f32)
            w2t = fp.tile([32, 64], f32)
            b2t = fp.tile([128, 64], f32)
            nc.sync.dma_start(w1t[:, :], w1t_d[:, :])
            nc.sync.dma_start(b1[:, :], b1_d[:, :])
            nc.sync.dma_start(w2t[:, :], w2t_d[:, :])
            nc.sync.dma_start(b2t[:, :], b2t_d[:, :])
            for h in range(4):
                ptsT = fp.tile([3, QT], f32, tag="ptsT")
                x1 = fp.tile([32, QT], f32, tag="x1")
                nc.sync.dma_start(ptsT[:, :], ptsT_d[:, h * QT:(h + 1) * QT])
                for j in range(QT // 512):
                    ps = pp.tile([32, 512], f32, tag="ps1")
                    nc.tensor.matmul(ps[:, :], w1t[:, :], ptsT[:, j * 512:(j + 1) * 512],
                                     start=True, stop=True)
                    nc.scalar.activation(x1[:, j * 512:(j + 1) * 512], ps[:, :],
                                         RELU, bias=b1[:, :], scale=1.0)
                for blk in range(QT // 128):
                    gblk = h * (QT // 128) + blk
                    if blk % 8 == 0:
                        stage = fb.tile([128, 8, 128], f16, tag="stage",
                                        name=f"stage_{h}_{blk}")
                    ps2 = pp.tile([128, 64], f32, tag="ps2")
                    nc.tensor.matmul(ps2[:, :], x1[:, blk * 128:(blk + 1) * 128],
                                     w2t[:, :], start=True, stop=True)
                    s2 = fb.tile([128, 64], f32, tag="s2")
                    nc.vector.tensor_tensor(s2[:, :], ps2[:, :], b2t[:, :], op=ADD)
                    nc.scalar.activation(stage[:, blk % 8, 0:64], s2[:, :], RELU)
                    if blk % 8 == 7:
                        g0 = (gblk - 7) * 128
                        nc.sync.dma_start(
                            tbl_d[g0:g0 + 1024, :].rearrange("(b p) c -> p b c", p=128),
                            stage[:, :, :])
            zr = fb.tile([1, 128], f16, tag="zr")
            nc.vector.memset(zr[:, :], 0)
            nc.sync.dma_start(tbl_d[ZERO_ID:ZERO_ID + 1, :], zr[:, :])

    # ---- TC2: gather image + conv stack ------------------------------------
    with TileContext(nc) as tc:
        with tc.tile_pool(name="cw", bufs=1) as wp, \
             tc.tile_pool(name="img", bufs=2) as ip, \
             tc.tile_pool(name="act", bufs=1) as ap_, \
             tc.tile_pool(name="sm", bufs=2) as sp, \
             tc.tile_pool(name="cps", bufs=4, space="PSUM") as pp:
            selw = wp.tile([128, NT * NZ * (GN // 16)], i16)
            nc.sync.dma_start(selw[:, :], selw_d[:, :])
            m0 = wp.tile([128, NT * C0WP], f16)
            mp = wp.tile([128, NT * P1WP], f16)
            c0w = wp.tile([64, 9, 64], f16)
            c0b = wp.tile([64, 1], f32)
            r1c1w = wp.tile([64, 9, 128], f16)
            r1c1b = wp.tile([128, 1], f32)
            r1c2w = wp.tile([128, 9, 128], f16)
            r1ob = wp.tile([128, 1], f32)
            r1scw = wp.tile([64, 128], f16)
            r2c1w = wp.tile([128, 9, 256], f16)
            r2c1b = wp.tile([128, 2], f32)
            r2c2w = wp.tile([128, 9, 2, 256], f16)
            r2ob = wp.tile([128, 2], f32)
            r2scw = wp.tile([128, 256], f16)
            nc.sync.dma_start(m0[:, :], m0_d[:, :])
            nc.sync.dma_start(mp[:, :], mp_d[:, :])
            nc.sync.dma_start(c0w[:, :, :], c0w_d[:, :, :])
            nc.sync.dma_start(c0b[:, :], c0b_d[:, :])
            nc.sync.dma_start(r1c1w[:, :, :], r1c1w_d[:, :, :])
            nc.sync.dma_start(r1c1b[:, :], r1c1b_d[:, :])
            nc.sync.dma_start(r1c2w[:, :, :], r1c2w_d[:, :, :])
            nc.sync.dma_start(r1ob[:, :], r1ob_d[:, :])
            nc.sync.dma_start(r1scw[:, :], r1scw_d[:, :])
            nc.sync.dma_start(r2c1w[:, :, :], r2c1w_d[:, :, :])
            nc.sync.dma_start(r2c1b[:, :], r2c1b_d[:, :])
            nc.sync.dma_start(r2c2w[:, :, :, :], r2c2w_d[:, :, :, :])
            nc.sync.dma_start(r2ob[:, :], r2ob_d[:, :])
            nc.sync.dma_start(r2scw[:, :], r2scw_d[:, :])

            def chunks(width, cmax=512):
                c = 0
                while c < width:
                    n = min(cmax, width - c)
                    yield c, n
                    c += n

            for t in range(NT):
                img = ip.tile([128, NZ, IMGW], f16, tag="img")
                for z in range(NZ):
                    nc.gpsimd.dma_gather(
                        out_ap=img[:, z:z + 1, :], in_ap=tbl_d[:, :],
                        idxs_ap=selw[:, (t * NZ + z) * (GN // 16):(t * NZ + z + 1) * (GN // 16)],
                        num_idxs=GN, num_idxs_reg=GN, elem_size=128,
                        transpose=True, single_packet=False)

                # conv0 s2 3x3 64->64, relu, mask -> c0o [64, 5, C0WP]
                c0o = ap_.tile([64, 5, C0WP], f16, tag="c0o")
                for ho in range(5):
                    for c, n in chunks(1112):
                        ps = pp.tile([64, 512], f32, tag="ps")
                        taps = [(dy, dx) for dy in (-1, 0, 1) for dx in (-1, 0, 1)
                                if 0 <= 2 * ho + dy < NZ]
                        for i, (dy, dx) in enumerate(taps):
                            st = 1 + dx + 2 * c
                            nc.tensor.matmul(
                                ps[:, :n], c0w[:, 3 * (dy + 1) + (dx + 1), :],
                                img[0:64, 2 * ho + dy, st:st + 2 * n:2],
                                start=(i == 0), stop=(i == len(taps) - 1))
                        nc.scalar.activation(c0o[:, ho, c:c + n], ps[:, :n],
                                             RELU, bias=c0b[:, :], scale=1.0)
                    nc.vector.tensor_tensor(c0o[:, ho, :], c0o[:, ho, :],
                                            m0[0:64, t * C0WP:(t + 1) * C0WP], op=MULT)

                # r1 h1 = relu(c1(c0o)), valid j in [1, 1111)
                h1 = ap_.tile([128, 5, C0WP], f16, tag="h1")
                for ho in range(5):
                    for c, n in chunks(1110):
                        ps = pp.tile([128, 512], f32, tag="ps")
                        taps = [(dy, dx) for dy in (-1, 0, 1) for dx in (-1, 0, 1)
                                if 0 <= ho + dy < 5]
                        for i, (dy, dx) in enumerate(taps):
                            st = 1 + c + dx
                            nc.tensor.matmul(
                                ps[:, :n], r1c1w[:, 3 * (dy + 1) + (dx + 1), :],
                                c0o[0:64, ho + dy, st:st + n],
                                start=(i == 0), stop=(i == len(taps) - 1))
                        nc.scalar.activation(h1[:, ho, 1 + c:1 + c + n], ps[:, :n],
                                             RELU, bias=r1c1b[:, :], scale=1.0)
                    nc.vector.tensor_tensor(h1[:, ho, :], h1[:, ho, :],
                                            m0[:, t * C0WP:(t + 1) * C0WP], op=MULT)

                # r1 out = relu(c2(h1) + sc(c0o)), valid j in [2, 1110)
                r1o = ap_.tile([128, 5, C0WP], f16, tag="r1o")
                for ho in range(5):
                    for c, n in chunks(1108):
                        ps = pp.tile([128, 512], f32, tag="ps")
                        taps = [(dy, dx) for dy in (-1, 0, 1) for dx in (-1, 0, 1)
                                if 0 <= ho + dy < 5]
                        for i, (dy, dx) in enumerate(taps):
                            st = 2 + c + dx
                            nc.tensor.matmul(
                                ps[:, :n], r1c2w[:, 3 * (dy + 1) + (dx + 1), :],
                                h1[:, ho + dy, st:st + n],
                                start=(i == 0), stop=False)
                        nc.tensor.matmul(ps[:, :n], r1scw[:, :],
                                         c0o[0:64, ho, 2 + c:2 + c + n],
                                         start=False, stop=True)
                        nc.scalar.activation(r1o[:, ho, 2 + c:2 + c + n], ps[:, :n],
                                             RELU, bias=r1ob[:, :], scale=1.0)

                # maxpool1 -> p1 [128, 2, P1WP] valid [0,554), mask
                p1 = ap_.tile([128, 2, P1WP], f16, tag="p1")
                tmpa = sp.tile([128, P1WP], f16, tag="tmpa")
                tmpb = sp.tile([128, P1WP], f16, tag="tmpb")
                for hp in range(2):
                    nc.vector.tensor_tensor(tmpa[:, 0:554],
                                            r1o[:, 2 * hp, 2:1110:2],
                                            r1o[:, 2 * hp, 3:1111:2], op=MAX)
                    nc.vector.tensor_tensor(tmpb[:, 0:554],
                                            r1o[:, 2 * hp + 1, 2:1110:2],
                                            r1o[:, 2 * hp + 1, 3:1111:2], op=MAX)
                    nc.vector.tensor_tensor(p1[:, hp, 0:554], tmpa[:, 0:554],
                                            tmpb[:, 0:554], op=MAX)
                    nc.vector.tensor_tensor(p1[:, hp, :], p1[:, hp, :],
                                            mp[:, t * P1WP:(t + 1) * P1WP], op=MULT)

                # r2 h2 = relu(c1(p1)), valid [1, 553), two M-halves
                h2 = [ap_.tile([128, 2, P1WP], f16, tag=f"h2{m}", name=f"h2{m}_{t}") for m in range(2)]
                for m in range(2):
                    for ho in range(2):
                        for c, n in chunks(552):
                            ps = pp.tile([128, 512], f32, tag="ps")
                            taps = [(dy, dx) for dy in (-1, 0, 1) for dx in (-1, 0, 1)
                                    if 0 <= ho + dy < 2]
                            for i, (dy, dx) in enumerate(taps):
                                st = 1 + c + dx
                                nc.tensor.matmul(
                                    ps[:, :n],
                                    r2c1w[:, 3 * (dy + 1) + (dx + 1), 128 * m:128 * (m + 1)],
                                    p1[:, ho + dy, st:st + n],
                                    start=(i == 0), stop=(i == len(taps) - 1))
                            nc.scalar.activation(h2[m][:, ho, 1 + c:1 + c + n], ps[:, :n],
                                                 RELU, bias=r2c1b[:, m:m + 1], scale=1.0)
                        nc.vector.tensor_tensor(h2[m][:, ho, :], h2[m][:, ho, :],
                                                mp[:, t * P1WP:(t + 1) * P1WP], op=MULT)

                # r2 out = relu(c2(h2) + sc(p1)), valid [2, 552)
                r2o = [ap_.tile([128, 2, P1WP], f16, tag=f"r2o{m}", name=f"r2o{m}_{t}") for m in range(2)]
                for m in range(2):
                    for ho in range(2):
                        for c, n in chunks(550):
                            ps = pp.tile([128, 512], f32, tag="ps")
                            taps = [(dy, dx) for dy in (-1, 0, 1) for dx in (-1, 0, 1)
                                    if 0 <= ho + dy < 2]
                            first = True
                            for (dy, dx) in taps:
                                for kh in range(2):
                                    st = 2 + c + dx
                                    nc.tensor.matmul(
                                        ps[:, :n],
                                        r2c2w[:, 3 * (dy + 1) + (dx + 1), kh, 128 * m:128 * (m + 1)],
                                        h2[kh][:, ho + dy, st:st + n],
                                        start=first, stop=False)
                                    first = False
                            nc.tensor.matmul(ps[:, :n], r2scw[:, 128 * m:128 * (m + 1)],
                                             p1[:, ho, 2 + c:2 + c + n],
                                             start=False, stop=True)
                            nc.scalar.activation(r2o[m][:, ho, 2 + c:2 + c + n], ps[:, :n],
                                                 RELU, bias=r2ob[:, m:m + 1], scale=1.0)

                # maxpool2 -> out [2x128, FW] f32
                for m in range(2):
                    oa = sp.tile([128, FW], f16, tag=f"oa{m}")
                    ob = sp.tile([128, FW], f16, tag=f"ob{m}")
                    oc = sp.tile([128, FW], f32, tag=f"oc{m}")
                    nc.vector.tensor_tensor(oa[:, :], r2o[m][:, 0, 2:552:2],
                                            r2o[m][:, 0, 3:553:2], op=MAX)
                    nc.vector.tensor_tensor(ob[:, :], r2o[m][:, 1, 2:552:2],
                                            r2o[m][:, 1, 3:553:2], op=MAX)
                    nc.vector.tensor_tensor(oc[:, :], oa[:, :], ob[:, :], op=MAX)
                    nc.sync.dma_start(out_d[128 * m:128 * (m + 1), t * FW:(t + 1) * FW],
                                      oc[:, :])
    nc.finalize()
    return nc


_NC_CACHE = None


def kernel(points, params):
    global _NC_CACHE, LAST_EXEC_NS
    from concourse.bass_utils import run_bass_kernel_spmd

    per_core, wd = _prep_host(points, params)
    if _NC_CACHE is None:
        _NC_CACHE = _build_bass()
    nc = _NC_CACHE
    in_maps = []
    for c in range(NCORES):
        m = dict(wd)
        m.update(per_core[c])
        in_maps.append({k: np.ascontiguousarray(v) for k, v in m.items()})
    trace = bool(int(os.environ.get("PP_TRACE", "0")))
    res = run_bass_kernel_spmd(nc, in_maps, core_ids=list(range(NCORES)),
                               trace=trace)
    LAST_EXEC_NS = res.exec_time_ns
    B = np.asarray(points).shape[0]
    y = np.zeros((B, 256, 1, 4400), np.float32)
    for c in range(NCORES):
        b, q = c // NQ, c % NQ
        y[b, :, 0, 1100 * q:1100 * (q + 1)] = res.results[c]['out']
    return y
